# revision 25
# baseline (speedup 1.0000x reference)
"""Causal multi-head attention block (qkv proj + attention + out proj) on 8
Trainium2 NeuronCores.

Sharding: Megatron-style tensor parallel over heads -- 2 heads per core.
Each core computes its heads' Q/K/V projections (column-sharded w_qkv),
causal attention for those heads, and a row-sharded partial of the output
projection.  The host sums the 8 partial outputs and adds b_o.

Device-side layout notes:
 - X^T [C, B*T] (fp16) feeds every matmul contraction dim on SBUF
   partitions with no on-device transposes.  Q^T/K^T come from the
   weight-stationary projection; V is produced keys-major directly by
   using the X^T tile as the stationary operand (out = X_chunk @ Wv), so
   no PE transpose or PSUM evacuation of V^T is needed.
 - Scores are computed transposed (S^T[k, q] = K^T.T @ Q^T per 128-wide
   k block) with the two heads' matmuls row-packed on the PE (partitions
   0:64 / 64:128).  Softmax exp runs on the scalar engine; the
   denominator is an extra all-ones column appended to V (row 64 of the
   attn@V accumulator).
 - The emission order interleaves the next batch's projections and the
   previous chunk's output projection into the attention kb-loop so the
   PE queue never drains (keeps the HAM clock-gate at 8/8).
 - Softmax divide: denominator rows are DMA-spread to 32 lanes,
   reciprocal on DVE, despread to one row, one gpsimd partition
   broadcast for both heads, then two DVE muls straight out of PSUM.
"""

import numpy as np
import ml_dtypes
from collections import deque
from contextlib import ExitStack

import concourse.bass as bass
import concourse.tile as tile
import concourse.mybir as mybir
from concourse import bacc
from concourse.bass_utils import run_bass_kernel_spmd

B, T, C, H, DH = 4, 2048, 1024, 16, 64
NCORES = 8
HPC = H // NCORES            # heads per core = 2
R = B * T                    # 8192 rows
HD = HPC * DH                # 128 local head dims
KT = C // 128                # 8 contraction tiles over C
RC = 512                     # row chunk in qkv stage
QC = 512                     # query chunk in attention
NQC = T // QC                # 4
NKB = T // 128               # 16 key blocks per batch

F32 = mybir.dt.float32
F16 = mybir.dt.float16

LAST_RESULT = None           # BassKernelResults of the most recent run
_CACHED_NC = None


def _emit(nc, tc, xt, wqkv, bqkv, wo, tri, y, use_bias=False):
    Exp = mybir.ActivationFunctionType.Exp
    with ExitStack() as ctx:
        const = ctx.enter_context(tc.tile_pool(name="const", bufs=1))
        bigp = ctx.enter_context(tc.tile_pool(name="bigp", bufs=2))
        xtp = ctx.enter_context(tc.tile_pool(name="xtp", bufs=3))
        vsbp = ctx.enter_context(tc.tile_pool(name="vsbp", bufs=2))
        ptp = ctx.enter_context(tc.tile_pool(name="ptp", bufs=5))
        osbp = ctx.enter_context(tc.tile_pool(name="osbp", bufs=2))
        ystp = ctx.enter_context(tc.tile_pool(name="ystp", bufs=4))
        smallp = ctx.enter_context(tc.tile_pool(name="smallp", bufs=2))
        psP = ctx.enter_context(tc.tile_pool(name="psP", bufs=2, space="PSUM"))
        psS = ctx.enter_context(tc.tile_pool(name="psS", bufs=2, space="PSUM"))
        psO = ctx.enter_context(tc.tile_pool(name="psO", bufs=1, space="PSUM"))

        # ---- constants (issued on idle queues so the sync queue can
        # start streaming x_t immediately) ----
        w_sb = const.tile([128, KT, 3 * HD], F16, name="w_sb")
        nc.scalar.dma_start(out=w_sb[:, :, :], in_=wqkv[:, :, :])
        wo_sb = const.tile([128, C], F16, name="wo_sb")
        nc.gpsimd.dma_start(out=wo_sb[:, :], in_=wo[:, :])
        b_sb = const.tile([128, 3], F32, name="b_sb")
        for m in range(3):
            nc.gpsimd.dma_start(
                out=b_sb[:, m : m + 1],
                in_=bqkv[m : m + 1, :].rearrange("a n -> n a"),
            )
        tri_sb = const.tile([128, 128], F16, name="tri_sb")
        nc.gpsimd.dma_start(out=tri_sb[:, :], in_=tri[:, :])
        bvv_sb = None
        if use_bias:
            bvv_sb = const.tile([128, HD], F32, name="bvv_sb")
            nc.sync.dma_start(
                out=bvv_sb[:, :],
                in_=bqkv[2:3, :].broadcast_to([128, HD]),
            )

        state = {}

        def alloc_batch(b):
            st = {
                "qt": bigp.tile([128, T], F16, name="qt", tag="qt"),
                "ktt": bigp.tile([128, T], F16, name="ktt", tag="ktt"),
                "vsb": vsbp.tile([128, NKB, 2, 65], F16, name="vsb", tag="vsb"),
                "osb": osbp.tile([128, T], F16, name="osb", tag="osb"),
                "xt": {},
            }
            return st

        def dma_unit(st, b2, rcl):
            def f():
                x_t = xtp.tile([128, KT, RC], F16, name="x_t", tag="xt")
                rc = b2 * (T // RC) + rcl
                # halves on two queues so the transfer finishes in half
                # the time (matters most for the prologue chunk)
                nc.sync.dma_start(
                    out=x_t[:, 0:4, :], in_=xt[:, rc, 0:4, :]
                )
                nc.gpsimd.dma_start(
                    out=x_t[:, 4:8, :], in_=xt[:, rc, 4:8, :]
                )
                st["xt"][rcl] = x_t

            return f

        def ones_unit(st):
            def f():
                nc.gpsimd.memset(st["vsb"][:, :, :, 64:65], 1.0)

            return f

        def qk_unit(st, rcl, m):
            def f():
                x_t = st["xt"][rcl]
                ps = psP.tile([128, RC], F32, name="ps_qk", tag="pp")
                for k in range(KT):
                    nc.tensor.matmul(
                        ps[:, :],
                        lhsT=w_sb[:, k, m * HD : (m + 1) * HD],
                        rhs=x_t[:, k, :],
                        start=(k == 0),
                        stop=(k == KT - 1),
                    )
                dst = (st["qt"] if m == 0 else st["ktt"])[
                    :, rcl * RC : (rcl + 1) * RC
                ]
                if use_bias:
                    nc.vector.tensor_scalar_add(
                        out=dst, in0=ps[:, :], scalar1=b_sb[:, m : m + 1]
                    )
                else:
                    nc.vector.tensor_copy(out=dst, in_=ps[:, :])

            return f

        def v_unit(st, rcl, rt):
            def f():
                x_t = st["xt"][rcl]
                ps = psP.tile([128, RC], F32, name="ps_v", tag="pp")
                for k in range(KT):
                    nc.tensor.matmul(
                        ps[:, 0:128],
                        lhsT=x_t[:, k, rt * 128 : (rt + 1) * 128],
                        rhs=w_sb[:, k, 2 * HD : 3 * HD],
                        start=(k == 0),
                        stop=(k == KT - 1),
                    )
                kb = rcl * 4 + rt
                vsb = st["vsb"]
                if use_bias:
                    # v bias varies along the free (dim) axis here, so a
                    # pre-replicated [128, HD] tile is added elementwise
                    nc.vector.tensor_add(
                        out=ps[:, 0:128], in0=ps[:, 0:128], in1=bvv_sb[:, :]
                    )
                nc.vector.tensor_copy(
                    out=vsb[:, kb, :, 0:64],
                    in_=ps[:, 0:128].rearrange("p (h d) -> p h d", h=2),
                )

            return f

        def stage_a_units(st, b2):
            us = [dma_unit(st, b2, 0), dma_unit(st, b2, 1), ones_unit(st)]
            for rcl in range(4):
                us.append(qk_unit(st, rcl, 0))
                if rcl + 2 < 4:
                    us.append(dma_unit(st, b2, rcl + 2))
                us.append(qk_unit(st, rcl, 1))
                for rt in range(4):
                    us.append(v_unit(st, rcl, rt))
            return us

        def oproj_unit(st, b, qc, half):
            def f():
                osb = st["osb"]
                for rb in (4 * qc + 2 * half, 4 * qc + 2 * half + 1):
                    for j in range(2):
                        ps = psP.tile([128, 512], F32, name="ps_o", tag="pp")
                        nc.tensor.matmul(
                            ps[:, :],
                            lhsT=osb[:, rb * 128 : (rb + 1) * 128],
                            rhs=wo_sb[:, j * 512 : (j + 1) * 512],
                            start=True,
                            stop=True,
                        )
                        yst = ystp.tile([128, 512], F16, name="yst", tag="yst")
                        nc.vector.tensor_copy(out=yst[:, :], in_=ps[:, :])
                        eng = nc.sync if (rb + j) % 2 == 0 else nc.gpsimd
                        eng.dma_start(
                            out=y[
                                b * T + rb * 128 : b * T + (rb + 1) * 128,
                                j * 512 : (j + 1) * 512,
                            ],
                            in_=yst[:, :],
                        )

            return f

        # two filler streams woven into the attention kb-loop:
        #  - aq: next batch's projections (independent, always ready)
        #  - lq: latency-sensitive deferred work (softmax-divide finish,
        #    output projection) that must not reach an engine queue before
        #    its upstream chain has had time to complete
        aq = deque()
        lq = deque()

        def pop_filler():
            if lq:
                lq.popleft()()
            elif aq:
                aq.popleft()()

        for b in range(B):
            if b == 0:
                state[0] = alloc_batch(0)
                for u in stage_a_units(state[0], 0):
                    u()
            if b + 1 < B:
                state[b + 1] = alloc_batch(b + 1)
                aq.extend(stage_a_units(state[b + 1], b + 1))

            st = state[b]
            qt, ktt, vsb, osb = st["qt"], st["ktt"], st["vsb"], st["osb"]

            for qc in range(NQC):
                o_ps = [
                    psO.tile([65, QC], F32, name=f"o_ps{h}", tag=f"o{h}")
                    for h in range(2)
                ]
                nkb = 4 * qc + 4

                def emit_av(kb, off, n, p_t):
                    for h in range(2):
                        nc.tensor.matmul(
                            o_ps[h][:, off:QC],
                            lhsT=vsb[:, kb, h, 0:65],
                            rhs=p_t[:, h, 0:n],
                            start=(kb == 0),
                            stop=(kb == nkb - 1),
                            skip_group_check=True,
                        )

                pending = []
                for kb in range(nkb):
                    off = max(0, (kb - 4 * qc) * 128)
                    n = QC - off
                    s_ps = psS.tile([128, 2, QC], F32, name="s_ps", tag="s")
                    for h in range(2):
                        nc.tensor.matmul(
                            s_ps[:, h, 0:n],
                            lhsT=ktt[
                                64 * h : 64 * h + 64,
                                kb * 128 : (kb + 1) * 128,
                            ],
                            rhs=qt[
                                64 * h : 64 * h + 64,
                                qc * QC + off : (qc + 1) * QC,
                            ],
                            start=True,
                            stop=True,
                        )
                    p_t = ptp.tile([128, 2, QC], F16, name="p_t", tag="pt")
                    nc.scalar.activation(
                        out=p_t[:, :, 0:n], in_=s_ps[:, :, 0:n], func=Exp
                    )
                    if kb >= 4 * qc:
                        nc.vector.tensor_mul(
                            out=p_t[:, :, 0:128],
                            in0=p_t[:, :, 0:128],
                            in1=tri_sb[:, :]
                            .unsqueeze(1)
                            .broadcast_to([128, 2, 128]),
                        )
                    # filler keeps the PE queue full while the exp for
                    # this block is still in flight
                    pop_filler()
                    pending.append((kb, off, n, p_t))
                    if len(pending) > 3:
                        emit_av(*pending.pop(0))
                for pv in pending:
                    emit_av(*pv)

                # ---- softmax divide: evacuate the accumulators to SBUF
                # right away (releases the PSUM banks so the next query
                # chunk's attn@V can start), then run the reciprocal
                # chain entirely out of SBUF.  The final muls are
                # deferred so they don't head-block the DVE queue while
                # the broadcast is still in flight.
                onum = [
                    smallp.tile([65, QC], F32, name=f"onum{h}", tag=f"on{h}")
                    for h in range(2)
                ]
                for h in range(2):
                    nc.vector.tensor_copy(out=onum[h][:, :], in_=o_ps[h][:, :])
                sp = smallp.tile([32, 32], F32, name="sp", tag="sp")
                nc.gpsimd.dma_start(out=sp[0:16, :], in_=onum[0][64:65, :])
                nc.gpsimd.dma_start(out=sp[16:32, :], in_=onum[1][64:65, :])
                sph = smallp.tile([32, 32], F16, name="sph", tag="sph")
                with nc.allow_low_precision(
                    reason="softmax reciprocal broadcast in fp16 is plenty"
                ):
                    nc.vector.reciprocal(out=sph[:, :], in_=sp[:, :])
                srow = smallp.tile([1, 2, QC], F16, name="srow", tag="srow")
                nc.gpsimd.dma_start(out=srow[0:1, :, :], in_=sph[:, :])
                bch = smallp.tile([64, 2, QC], F16, name="bch", tag="bch")
                nc.gpsimd.partition_broadcast(
                    out_ap=bch[:, :, :], in_ap=srow[0:1, :, :]
                )

                def div_fin(qc=qc, onum=onum, bch=bch, osb=osb):
                    nc.vector.tensor_mul(
                        out=osb[0:64, qc * QC : (qc + 1) * QC],
                        in0=onum[0][0:64, :],
                        in1=bch[:, 0, :],
                    )
                    htmp = smallp.tile([64, QC], F16, name="htmp", tag="htmp")
                    nc.vector.tensor_mul(
                        out=htmp[:, :], in0=onum[1][0:64, :], in1=bch[:, 1, :]
                    )
                    nc.sync.dma_start(
                        out=osb[64:128, qc * QC : (qc + 1) * QC],
                        in_=htmp[:, :],
                    )

                if qc >= 1:
                    lq.append(oproj_unit(st, b, qc - 1, 0))
                    lq.append(oproj_unit(st, b, qc - 1, 1))
                lq.append(div_fin)

            # drain the projection stream for the next batch; carry the
            # last query chunk's divide/oproj into the next batch's slots
            while aq:
                aq.popleft()()
            lq.append(oproj_unit(st, b, NQC - 1, 0))
            lq.append(oproj_unit(st, b, NQC - 1, 1))
            if b == B - 1:
                while lq:
                    lq.popleft()()
            if b - 1 in state:
                del state[b - 1]


def _build(use_bias=False):
    nc = bacc.Bacc("TRN2", target_bir_lowering=False)
    xt = nc.dram_tensor("xt", [128, R // RC, KT, RC], F16, kind="ExternalInput")
    wqkv = nc.dram_tensor("wqkv", [128, KT, 3 * HD], F16, kind="ExternalInput")
    bqkv = nc.dram_tensor("bqkv", [3, HD], F32, kind="ExternalInput")
    wo = nc.dram_tensor("wo", [HD, C], F16, kind="ExternalInput")
    tri = nc.dram_tensor("tri", [128, 128], F16, kind="ExternalInput")
    y = nc.dram_tensor("y", [R, C], F16, kind="ExternalOutput")
    with tile.TileContext(nc) as tc:
        _emit(nc, tc, xt, wqkv, bqkv, wo, tri, y, use_bias)
    nc.finalize()
    return nc


def kernel(hidden_states, w_qkv, b_qkv, w_o, b_o):
    global LAST_RESULT, _CACHED_NC
    X = np.ascontiguousarray(np.asarray(hidden_states, dtype=np.float32)).reshape(
        R, C
    )
    w_qkv = np.asarray(w_qkv, dtype=np.float32)
    b_qkv = np.asarray(b_qkv, dtype=np.float32)
    w_o = np.asarray(w_o, dtype=np.float32)
    b_o = np.asarray(b_o, dtype=np.float32)

    # [ki, rc, ko, col] layout: each partition's per-chunk read is one
    # contiguous 8 KB run, so the x_t DMAs stream at full bandwidth
    Xt = X.T.astype(np.float16).reshape(KT, 128, R // RC, RC)
    Xt = np.ascontiguousarray(Xt.transpose(1, 2, 0, 3))
    scale = float(DH) ** -0.5
    tri_m = np.triu(np.ones((128, 128), dtype=np.float32)).astype(np.float16)

    in_maps = []
    for c in range(NCORES):
        heads = [HPC * c + i for i in range(HPC)]
        wcols, bcols = [], []
        for sec in range(3):  # q, k, v
            sc = scale if sec == 0 else 1.0
            for h in heads:
                lo = sec * C + h * DH
                wcols.append(w_qkv[:, lo : lo + DH] * sc)
                bcols.append(b_qkv[lo : lo + DH] * sc)
        wqkv_c = (
            np.concatenate(wcols, axis=1)
            .astype(np.float16)
            .reshape(KT, 128, 3 * HD)
        )
        wqkv_c = np.ascontiguousarray(wqkv_c.transpose(1, 0, 2))
        bqkv_c = np.ascontiguousarray(np.concatenate(bcols).reshape(3, HD))
        wo_c = np.ascontiguousarray(
            np.concatenate([w_o[h * DH : (h + 1) * DH, :] for h in heads], axis=0)
        ).astype(np.float16)  # [HD, C]
        in_maps.append(
            {
                "xt": Xt,
                "wqkv": wqkv_c,
                "bqkv": bqkv_c,
                "wo": wo_c,
                "tri": tri_m,
            }
        )

    if _CACHED_NC is None:
        _CACHED_NC = _build(use_bias=bool(np.any(b_qkv)))
    res = run_bass_kernel_spmd(_CACHED_NC, in_maps, core_ids=list(range(NCORES)))
    LAST_RESULT = res

    out = res.results[0]["y"].astype(np.float64)
    for c in range(1, NCORES):
        out += res.results[c]["y"]
    out += b_o
    return out.astype(np.float32).reshape(B, T, C)


# revision 27
# speedup vs baseline: 1.0145x; 1.0145x over previous
"""Causal multi-head attention block (qkv proj + attention + out proj) on 8
Trainium2 NeuronCores.

Sharding: Megatron-style tensor parallel over heads -- 2 heads per core.
Each core computes its heads' Q/K/V projections (column-sharded w_qkv),
causal attention for those heads, and a row-sharded partial of the output
projection.  The host sums the 8 partial outputs and adds b_o.

Device-side layout notes:
 - X^T [C, B*T] (fp16) feeds every matmul contraction dim on SBUF
   partitions with no on-device transposes.  Q^T/K^T come from the
   weight-stationary projection; V is produced keys-major directly by
   using the X^T tile as the stationary operand (out = X_chunk @ Wv), so
   no PE transpose or PSUM evacuation of V^T is needed.
 - Scores are computed transposed (S^T[k, q] = K^T.T @ Q^T per 128-wide
   k block) with the two heads' matmuls row-packed on the PE (partitions
   0:64 / 64:128).  Softmax exp runs on the scalar engine; the
   denominator is an extra all-ones column appended to V (row 64 of the
   attn@V accumulator).
 - The emission order interleaves the next batch's projections and the
   previous chunk's output projection into the attention kb-loop so the
   PE queue never drains (keeps the HAM clock-gate at 8/8).
 - Softmax divide: denominator rows are DMA-spread to 32 lanes,
   reciprocal on DVE, despread to one row, one gpsimd partition
   broadcast for both heads, then two DVE muls straight out of PSUM.
"""

import numpy as np
import ml_dtypes
from collections import deque
from contextlib import ExitStack

import concourse.bass as bass
import concourse.tile as tile
import concourse.mybir as mybir
from concourse import bacc
from concourse.bass_utils import run_bass_kernel_spmd

B, T, C, H, DH = 4, 2048, 1024, 16, 64
NCORES = 8
HPC = H // NCORES            # heads per core = 2
R = B * T                    # 8192 rows
HD = HPC * DH                # 128 local head dims
KT = C // 128                # 8 contraction tiles over C
RC = 512                     # row chunk in qkv stage
QC = 512                     # query chunk in attention
NQC = T // QC                # 4
NKB = T // 128               # 16 key blocks per batch

F32 = mybir.dt.float32
F16 = mybir.dt.float16

LAST_RESULT = None           # BassKernelResults of the most recent run
_CACHED_NC = None


def _emit(nc, tc, xt, wqkv, bqkv, wo, tri, y, use_bias=False):
    Exp = mybir.ActivationFunctionType.Exp
    with ExitStack() as ctx:
        const = ctx.enter_context(tc.tile_pool(name="const", bufs=1))
        bigp = ctx.enter_context(tc.tile_pool(name="bigp", bufs=2))
        xtp = ctx.enter_context(tc.tile_pool(name="xtp", bufs=3))
        vsbp = ctx.enter_context(tc.tile_pool(name="vsbp", bufs=2))
        ptp = ctx.enter_context(tc.tile_pool(name="ptp", bufs=5))
        osbp = ctx.enter_context(tc.tile_pool(name="osbp", bufs=2))
        ystp = ctx.enter_context(tc.tile_pool(name="ystp", bufs=4))
        smallp = ctx.enter_context(tc.tile_pool(name="smallp", bufs=2))
        psP = ctx.enter_context(tc.tile_pool(name="psP", bufs=2, space="PSUM"))
        psS = ctx.enter_context(tc.tile_pool(name="psS", bufs=2, space="PSUM"))
        psO = ctx.enter_context(tc.tile_pool(name="psO", bufs=1, space="PSUM"))

        # ---- constants (issued on idle queues so the sync queue can
        # start streaming x_t immediately) ----
        w_sb = const.tile([128, KT, 3 * HD], F16, name="w_sb")
        nc.scalar.dma_start(out=w_sb[:, :, :], in_=wqkv[:, :, :])
        wo_sb = const.tile([128, C], F16, name="wo_sb")
        nc.gpsimd.dma_start(out=wo_sb[:, :], in_=wo[:, :])
        b_sb = const.tile([128, 3], F32, name="b_sb")
        for m in range(3):
            nc.gpsimd.dma_start(
                out=b_sb[:, m : m + 1],
                in_=bqkv[m : m + 1, :].rearrange("a n -> n a"),
            )
        tri_sb = const.tile([128, 128], F16, name="tri_sb")
        nc.gpsimd.dma_start(out=tri_sb[:, :], in_=tri[:, :])
        bvv_sb = None
        if use_bias:
            bvv_sb = const.tile([128, HD], F32, name="bvv_sb")
            nc.sync.dma_start(
                out=bvv_sb[:, :],
                in_=bqkv[2:3, :].broadcast_to([128, HD]),
            )

        state = {}

        def alloc_batch(b):
            st = {
                "qt": bigp.tile([128, T], F16, name="qt", tag="qt"),
                "ktt": bigp.tile([128, T], F16, name="ktt", tag="ktt"),
                "vsb": vsbp.tile([128, NKB, 2, 65], F16, name="vsb", tag="vsb"),
                "osb": osbp.tile([128, T], F16, name="osb", tag="osb"),
                "xt": {},
            }
            return st

        def dma_unit(st, b2, rcl):
            def f():
                x_t = xtp.tile([128, KT, RC], F16, name="x_t", tag="xt")
                rc = b2 * (T // RC) + rcl
                nc.sync.dma_start(out=x_t[:, :, :], in_=xt[:, rc, :, :])
                st["xt"][rcl] = x_t

            return f

        def ones_unit(st):
            def f():
                nc.gpsimd.memset(st["vsb"][:, :, :, 64:65], 1.0)

            return f

        def qk_unit(st, rcl, m):
            def f():
                x_t = st["xt"][rcl]
                ps = psP.tile([128, RC], F32, name="ps_qk", tag="pp")
                for k in range(KT):
                    nc.tensor.matmul(
                        ps[:, :],
                        lhsT=w_sb[:, k, m * HD : (m + 1) * HD],
                        rhs=x_t[:, k, :],
                        start=(k == 0),
                        stop=(k == KT - 1),
                    )
                dst = (st["qt"] if m == 0 else st["ktt"])[
                    :, rcl * RC : (rcl + 1) * RC
                ]
                if use_bias:
                    nc.vector.tensor_scalar_add(
                        out=dst, in0=ps[:, :], scalar1=b_sb[:, m : m + 1]
                    )
                else:
                    nc.vector.tensor_copy(out=dst, in_=ps[:, :])

            return f

        def v_unit(st, rcl, rt):
            def f():
                x_t = st["xt"][rcl]
                ps = psP.tile([128, RC], F32, name="ps_v", tag="pp")
                for k in range(KT):
                    nc.tensor.matmul(
                        ps[:, 0:128],
                        lhsT=x_t[:, k, rt * 128 : (rt + 1) * 128],
                        rhs=w_sb[:, k, 2 * HD : 3 * HD],
                        start=(k == 0),
                        stop=(k == KT - 1),
                    )
                kb = rcl * 4 + rt
                vsb = st["vsb"]
                if use_bias:
                    # v bias varies along the free (dim) axis here, so a
                    # pre-replicated [128, HD] tile is added elementwise
                    nc.vector.tensor_add(
                        out=ps[:, 0:128], in0=ps[:, 0:128], in1=bvv_sb[:, :]
                    )
                nc.vector.tensor_copy(
                    out=vsb[:, kb, :, 0:64],
                    in_=ps[:, 0:128].rearrange("p (h d) -> p h d", h=2),
                )

            return f

        def stage_a_units(st, b2):
            us = [dma_unit(st, b2, 0), dma_unit(st, b2, 1), ones_unit(st)]
            for rcl in range(4):
                us.append(qk_unit(st, rcl, 0))
                if rcl + 2 < 4:
                    us.append(dma_unit(st, b2, rcl + 2))
                us.append(qk_unit(st, rcl, 1))
                for rt in range(4):
                    us.append(v_unit(st, rcl, rt))
            return us

        def oproj_unit(st, b, qc, half):
            def f():
                osb = st["osb"]
                for rb in (4 * qc + 2 * half, 4 * qc + 2 * half + 1):
                    for j in range(2):
                        ps = psP.tile([128, 512], F32, name="ps_o", tag="pp")
                        nc.tensor.matmul(
                            ps[:, :],
                            lhsT=osb[:, rb * 128 : (rb + 1) * 128],
                            rhs=wo_sb[:, j * 512 : (j + 1) * 512],
                            start=True,
                            stop=True,
                        )
                        yst = ystp.tile([128, 512], F16, name="yst", tag="yst")
                        nc.vector.tensor_copy(out=yst[:, :], in_=ps[:, :])
                        nc.sync.dma_start(
                            out=y[
                                b * T + rb * 128 : b * T + (rb + 1) * 128,
                                j * 512 : (j + 1) * 512,
                            ],
                            in_=yst[:, :],
                        )

            return f

        # two filler streams woven into the attention kb-loop:
        #  - aq: next batch's projections (independent, always ready)
        #  - lq: latency-sensitive deferred work (softmax-divide finish,
        #    output projection) that must not reach an engine queue before
        #    its upstream chain has had time to complete
        aq = deque()
        lq = deque()

        def pop_filler():
            if lq:
                lq.popleft()()
            elif aq:
                aq.popleft()()

        for b in range(B):
            if b == 0:
                state[0] = alloc_batch(0)
                for u in stage_a_units(state[0], 0):
                    u()
            if b + 1 < B:
                state[b + 1] = alloc_batch(b + 1)
                aq.extend(stage_a_units(state[b + 1], b + 1))

            st = state[b]
            qt, ktt, vsb, osb = st["qt"], st["ktt"], st["vsb"], st["osb"]

            for qc in range(NQC):
                o_ps = [
                    psO.tile([65, QC], F32, name=f"o_ps{h}", tag=f"o{h}")
                    for h in range(2)
                ]
                nkb = 4 * qc + 4

                def emit_av(kb, off, n, p_t):
                    for h in range(2):
                        nc.tensor.matmul(
                            o_ps[h][:, off:QC],
                            lhsT=vsb[:, kb, h, 0:65],
                            rhs=p_t[:, h, 0:n],
                            start=(kb == 0),
                            stop=(kb == nkb - 1),
                            skip_group_check=True,
                        )

                pending = []
                for kb in range(nkb):
                    off = max(0, (kb - 4 * qc) * 128)
                    n = QC - off
                    s_ps = psS.tile([128, 2, QC], F32, name="s_ps", tag="s")
                    for h in range(2):
                        nc.tensor.matmul(
                            s_ps[:, h, 0:n],
                            lhsT=ktt[
                                64 * h : 64 * h + 64,
                                kb * 128 : (kb + 1) * 128,
                            ],
                            rhs=qt[
                                64 * h : 64 * h + 64,
                                qc * QC + off : (qc + 1) * QC,
                            ],
                            start=True,
                            stop=True,
                        )
                    p_t = ptp.tile([128, 2, QC], F16, name="p_t", tag="pt")
                    nc.scalar.activation(
                        out=p_t[:, :, 0:n], in_=s_ps[:, :, 0:n], func=Exp
                    )
                    if kb >= 4 * qc:
                        nc.vector.tensor_mul(
                            out=p_t[:, :, 0:128],
                            in0=p_t[:, :, 0:128],
                            in1=tri_sb[:, :]
                            .unsqueeze(1)
                            .broadcast_to([128, 2, 128]),
                        )
                    # filler keeps the PE queue full while the exp for
                    # this block is still in flight
                    pop_filler()
                    pending.append((kb, off, n, p_t))
                    if len(pending) > 3:
                        emit_av(*pending.pop(0))
                for pv in pending:
                    emit_av(*pv)

                # ---- softmax divide: evacuate the accumulators to SBUF
                # right away (releases the PSUM banks so the next query
                # chunk's attn@V can start), then run the reciprocal
                # chain entirely out of SBUF.  The final muls are
                # deferred so they don't head-block the DVE queue while
                # the broadcast is still in flight.
                onum = [
                    smallp.tile([65, QC], F32, name=f"onum{h}", tag=f"on{h}")
                    for h in range(2)
                ]
                for h in range(2):
                    nc.vector.tensor_copy(out=onum[h][:, :], in_=o_ps[h][:, :])
                sp = smallp.tile([32, 32], F32, name="sp", tag="sp")
                nc.gpsimd.dma_start(out=sp[0:16, :], in_=onum[0][64:65, :])
                nc.gpsimd.dma_start(out=sp[16:32, :], in_=onum[1][64:65, :])
                sph = smallp.tile([32, 32], F16, name="sph", tag="sph")
                with nc.allow_low_precision(
                    reason="softmax reciprocal broadcast in fp16 is plenty"
                ):
                    nc.vector.reciprocal(out=sph[:, :], in_=sp[:, :])
                srow = smallp.tile([1, 2, QC], F16, name="srow", tag="srow")
                nc.gpsimd.dma_start(out=srow[0:1, :, :], in_=sph[:, :])
                bch = smallp.tile([64, 2, QC], F16, name="bch", tag="bch")
                nc.gpsimd.partition_broadcast(
                    out_ap=bch[:, :, :], in_ap=srow[0:1, :, :]
                )

                def div_fin(qc=qc, onum=onum, bch=bch, osb=osb):
                    nc.vector.tensor_mul(
                        out=osb[0:64, qc * QC : (qc + 1) * QC],
                        in0=onum[0][0:64, :],
                        in1=bch[:, 0, :],
                    )
                    htmp = smallp.tile([64, QC], F16, name="htmp", tag="htmp")
                    nc.vector.tensor_mul(
                        out=htmp[:, :], in0=onum[1][0:64, :], in1=bch[:, 1, :]
                    )
                    nc.sync.dma_start(
                        out=osb[64:128, qc * QC : (qc + 1) * QC],
                        in_=htmp[:, :],
                    )

                if qc >= 1:
                    lq.append(oproj_unit(st, b, qc - 1, 0))
                    lq.append(oproj_unit(st, b, qc - 1, 1))
                lq.append(div_fin)

            # drain the projection stream for the next batch; carry the
            # last query chunk's divide/oproj into the next batch's slots
            while aq:
                aq.popleft()()
            lq.append(oproj_unit(st, b, NQC - 1, 0))
            lq.append(oproj_unit(st, b, NQC - 1, 1))
            if b == B - 1:
                while lq:
                    lq.popleft()()
            if b - 1 in state:
                del state[b - 1]


def _build(use_bias=False):
    nc = bacc.Bacc("TRN2", target_bir_lowering=False)
    xt = nc.dram_tensor("xt", [128, R // RC, KT, RC], F16, kind="ExternalInput")
    wqkv = nc.dram_tensor("wqkv", [128, KT, 3 * HD], F16, kind="ExternalInput")
    bqkv = nc.dram_tensor("bqkv", [3, HD], F32, kind="ExternalInput")
    wo = nc.dram_tensor("wo", [HD, C], F16, kind="ExternalInput")
    tri = nc.dram_tensor("tri", [128, 128], F16, kind="ExternalInput")
    y = nc.dram_tensor("y", [R, C], F16, kind="ExternalOutput")
    with tile.TileContext(nc) as tc:
        _emit(nc, tc, xt, wqkv, bqkv, wo, tri, y, use_bias)
    nc.finalize()
    return nc


def kernel(hidden_states, w_qkv, b_qkv, w_o, b_o):
    global LAST_RESULT, _CACHED_NC
    X = np.ascontiguousarray(np.asarray(hidden_states, dtype=np.float32)).reshape(
        R, C
    )
    w_qkv = np.asarray(w_qkv, dtype=np.float32)
    b_qkv = np.asarray(b_qkv, dtype=np.float32)
    w_o = np.asarray(w_o, dtype=np.float32)
    b_o = np.asarray(b_o, dtype=np.float32)

    # [ki, rc, ko, col] layout: each partition's per-chunk read is one
    # contiguous 8 KB run, so the x_t DMAs stream at full bandwidth
    Xt = X.T.astype(np.float16).reshape(KT, 128, R // RC, RC)
    Xt = np.ascontiguousarray(Xt.transpose(1, 2, 0, 3))
    scale = float(DH) ** -0.5
    tri_m = np.triu(np.ones((128, 128), dtype=np.float32)).astype(np.float16)

    in_maps = []
    for c in range(NCORES):
        heads = [HPC * c + i for i in range(HPC)]
        wcols, bcols = [], []
        for sec in range(3):  # q, k, v
            sc = scale if sec == 0 else 1.0
            for h in heads:
                lo = sec * C + h * DH
                wcols.append(w_qkv[:, lo : lo + DH] * sc)
                bcols.append(b_qkv[lo : lo + DH] * sc)
        wqkv_c = (
            np.concatenate(wcols, axis=1)
            .astype(np.float16)
            .reshape(KT, 128, 3 * HD)
        )
        wqkv_c = np.ascontiguousarray(wqkv_c.transpose(1, 0, 2))
        bqkv_c = np.ascontiguousarray(np.concatenate(bcols).reshape(3, HD))
        wo_c = np.ascontiguousarray(
            np.concatenate([w_o[h * DH : (h + 1) * DH, :] for h in heads], axis=0)
        ).astype(np.float16)  # [HD, C]
        in_maps.append(
            {
                "xt": Xt,
                "wqkv": wqkv_c,
                "bqkv": bqkv_c,
                "wo": wo_c,
                "tri": tri_m,
            }
        )

    if _CACHED_NC is None:
        _CACHED_NC = _build(use_bias=bool(np.any(b_qkv)))
    res = run_bass_kernel_spmd(_CACHED_NC, in_maps, core_ids=list(range(NCORES)))
    LAST_RESULT = res

    out = res.results[0]["y"].astype(np.float64)
    for c in range(1, NCORES):
        out += res.results[c]["y"]
    out += b_o
    return out.astype(np.float32).reshape(B, T, C)


# revision 34
# speedup vs baseline: 1.1697x; 1.1530x over previous
"""Causal multi-head attention block (qkv proj + attention + out proj) on 8
Trainium2 NeuronCores.

Sharding: Megatron-style tensor parallel over heads -- 2 heads per core.
Each core computes its heads' Q/K/V projections (column-sharded w_qkv),
causal attention for those heads, and a row-sharded partial of the output
projection.  The host sums the 8 partial outputs and adds b_o.

Device-side layout notes:
 - X^T [C, B*T] (fp16) feeds every matmul contraction dim on SBUF
   partitions with no on-device transposes.  Q^T/K^T come from the
   weight-stationary projection; V is produced keys-major directly by
   using the X^T tile as the stationary operand (out = X_chunk @ Wv), so
   no PE transpose or PSUM evacuation of V^T is needed.
 - Scores are computed transposed (S^T[k, q] = K^T.T @ Q^T per 128-wide
   k block) with the two heads' matmuls row-packed on the PE (partitions
   0:64 / 64:128).  Softmax exp runs on the scalar engine; the
   denominator is an extra all-ones column appended to V (row 64 of the
   attn@V accumulator).
 - The emission order interleaves the next batch's projections and the
   previous chunk's output projection into the attention kb-loop so the
   PE queue never drains (keeps the HAM clock-gate at 8/8).
 - Softmax divide: denominator rows are DMA-spread to 32 lanes,
   reciprocal on DVE, despread to one row, one gpsimd partition
   broadcast for both heads, then two DVE muls straight out of PSUM.
"""

import numpy as np
import ml_dtypes
from collections import deque
from contextlib import ExitStack

import concourse.bass as bass
import concourse.tile as tile
import concourse.mybir as mybir
from concourse import bacc
from concourse.bass_utils import run_bass_kernel_spmd

B, T, C, H, DH = 4, 2048, 1024, 16, 64
NCORES = 8
HPC = H // NCORES            # heads per core = 2
R = B * T                    # 8192 rows
HD = HPC * DH                # 128 local head dims
KT = C // 128                # 8 contraction tiles over C
RC = 512                     # row chunk in qkv stage
QC = 512                     # query chunk in attention
NQC = T // QC                # 4
NKB = T // 128               # 16 key blocks per batch

F32 = mybir.dt.float32
F16 = mybir.dt.float16

LAST_RESULT = None           # BassKernelResults of the most recent run
_CACHED_NC = None


def _emit(nc, tc, xt, wqkv, bqkv, wo, tri, y, use_bias=False):
    Exp = mybir.ActivationFunctionType.Exp
    with ExitStack() as ctx:
        const = ctx.enter_context(tc.tile_pool(name="const", bufs=1))
        bigp = ctx.enter_context(tc.tile_pool(name="bigp", bufs=2))
        xtp = ctx.enter_context(tc.tile_pool(name="xtp", bufs=3))
        vsbp = ctx.enter_context(tc.tile_pool(name="vsbp", bufs=2))
        ptp = ctx.enter_context(tc.tile_pool(name="ptp", bufs=5))
        osbp = ctx.enter_context(tc.tile_pool(name="osbp", bufs=2))
        ystp = ctx.enter_context(tc.tile_pool(name="ystp", bufs=4))
        smallp = ctx.enter_context(tc.tile_pool(name="smallp", bufs=2))
        psP = ctx.enter_context(tc.tile_pool(name="psP", bufs=2, space="PSUM"))
        psS = ctx.enter_context(tc.tile_pool(name="psS", bufs=2, space="PSUM"))
        psO = ctx.enter_context(tc.tile_pool(name="psO", bufs=1, space="PSUM"))

        # ---- constants (issued on idle queues so the sync queue can
        # start streaming x_t immediately) ----
        w_sb = const.tile([128, KT, 3 * HD], F16, name="w_sb")
        for m in range(3):
            nc.scalar.dma_start(
                out=w_sb[:, :, m * HD : (m + 1) * HD],
                in_=wqkv[:, :, m * HD : (m + 1) * HD],
            )
        wo_sb = const.tile([128, C], F16, name="wo_sb")
        nc.gpsimd.dma_start(out=wo_sb[:, :], in_=wo[:, :])
        b_sb = const.tile([128, 3], F32, name="b_sb")
        for m in range(3):
            nc.gpsimd.dma_start(
                out=b_sb[:, m : m + 1],
                in_=bqkv[m : m + 1, :].rearrange("a n -> n a"),
            )
        tri_sb = const.tile([128, 128], F16, name="tri_sb")
        nc.gpsimd.dma_start(out=tri_sb[:, :], in_=tri[:, :])
        bvv_sb = None
        if use_bias:
            bvv_sb = const.tile([128, HD], F32, name="bvv_sb")
            nc.sync.dma_start(
                out=bvv_sb[:, :],
                in_=bqkv[2:3, :].broadcast_to([128, HD]),
            )

        state = {}

        def alloc_batch(b):
            st = {
                "qt": bigp.tile([128, T], F16, name="qt", tag="qt"),
                "ktt": bigp.tile([128, T], F16, name="ktt", tag="ktt"),
                "vsb": vsbp.tile([128, NKB, 2, 65], F16, name="vsb", tag="vsb"),
                "osb": osbp.tile([128, T], F16, name="osb", tag="osb"),
                "xt": {},
            }
            return st

        def dma_unit(st, b2, rcl):
            def f():
                x_t = xtp.tile([128, KT, RC], F16, name="x_t", tag="xt")
                rc = b2 * (T // RC) + rcl
                nc.sync.dma_start(out=x_t[:, :, :], in_=xt[:, rc, :, :])
                st["xt"][rcl] = x_t

            return f

        def ones_unit(st):
            def f():
                nc.gpsimd.memset(st["vsb"][:, :, :, 64:65], 1.0)

            return f

        def qk_unit(st, rcl, m):
            def f():
                x_t = st["xt"][rcl]
                ps = psP.tile([128, RC], F32, name="ps_qk", tag="pp")
                for k in range(KT):
                    nc.tensor.matmul(
                        ps[:, :],
                        lhsT=w_sb[:, k, m * HD : (m + 1) * HD],
                        rhs=x_t[:, k, :],
                        start=(k == 0),
                        stop=(k == KT - 1),
                    )
                dst = (st["qt"] if m == 0 else st["ktt"])[
                    :, rcl * RC : (rcl + 1) * RC
                ]
                if use_bias:
                    nc.vector.tensor_scalar_add(
                        out=dst, in0=ps[:, :], scalar1=b_sb[:, m : m + 1]
                    )
                else:
                    nc.vector.tensor_copy(out=dst, in_=ps[:, :])

            return f

        def v_unit(st, rcl, rt):
            def f():
                x_t = st["xt"][rcl]
                ps = psP.tile([128, RC], F32, name="ps_v", tag="pp")
                for k in range(KT):
                    nc.tensor.matmul(
                        ps[:, 0:128],
                        lhsT=x_t[:, k, rt * 128 : (rt + 1) * 128],
                        rhs=w_sb[:, k, 2 * HD : 3 * HD],
                        start=(k == 0),
                        stop=(k == KT - 1),
                    )
                kb = rcl * 4 + rt
                vsb = st["vsb"]
                if use_bias:
                    # v bias varies along the free (dim) axis here, so a
                    # pre-replicated [128, HD] tile is added elementwise
                    nc.vector.tensor_add(
                        out=ps[:, 0:128], in0=ps[:, 0:128], in1=bvv_sb[:, :]
                    )
                nc.vector.tensor_copy(
                    out=vsb[:, kb, :, 0:64],
                    in_=ps[:, 0:128].rearrange("p (h d) -> p h d", h=2),
                )

            return f

        def stage_a_units(st, b2, prologue=False):
            if prologue:
                # first batch is gated on its own first chunk: don't put
                # prefetches ahead of it in the DMA engines
                us = [dma_unit(st, b2, 0), ones_unit(st)]
                for rcl in range(4):
                    us.append(qk_unit(st, rcl, 0))
                    if rcl + 1 < 4:
                        us.append(dma_unit(st, b2, rcl + 1))
                    us.append(qk_unit(st, rcl, 1))
                    for rt in range(4):
                        us.append(v_unit(st, rcl, rt))
                return us
            us = [dma_unit(st, b2, 0), dma_unit(st, b2, 1), ones_unit(st)]
            for rcl in range(4):
                us.append(qk_unit(st, rcl, 0))
                if rcl + 2 < 4:
                    us.append(dma_unit(st, b2, rcl + 2))
                us.append(qk_unit(st, rcl, 1))
                for rt in range(4):
                    us.append(v_unit(st, rcl, rt))
            return us

        def oproj_unit(st, b, qc, half, split_q=False):
            def f():
                osb = st["osb"]
                for rb in (4 * qc + 2 * half, 4 * qc + 2 * half + 1):
                    for j in range(2):
                        ps = psP.tile([128, 512], F32, name="ps_o", tag="pp")
                        nc.tensor.matmul(
                            ps[:, :],
                            lhsT=osb[:, rb * 128 : (rb + 1) * 128],
                            rhs=wo_sb[:, j * 512 : (j + 1) * 512],
                            start=True,
                            stop=True,
                        )
                        yst = ystp.tile([128, 512], F16, name="yst", tag="yst")
                        nc.vector.tensor_copy(out=yst[:, :], in_=ps[:, :])
                        eng = nc.scalar if (split_q and j == 1) else nc.sync
                        eng.dma_start(
                            out=y[
                                b * T + rb * 128 : b * T + (rb + 1) * 128,
                                j * 512 : (j + 1) * 512,
                            ],
                            in_=yst[:, :],
                        )

            return f

        # two filler streams woven into the attention kb-loop:
        #  - aq: next batch's projections (independent, always ready)
        #  - lq: latency-sensitive deferred work (softmax-divide finish,
        #    output projection) that must not reach an engine queue before
        #    its upstream chain has had time to complete
        aq = deque()
        lq = deque()  # entries: (kind, fn); 'df' = divide-finish, 'op' = oproj
        RESERVE = 5   # aq units held back to cushion the batch-end chain

        def pop_filler(slot):
            # divide-finish muls must not hit the DVE queue before their
            # broadcast has had ~2 slots to complete
            if lq and (lq[0][0] != "df" or slot >= 2):
                lq.popleft()[1]()
            elif len(aq) > RESERVE:
                aq.popleft()()

        for b in range(B):
            if b == 0:
                state[0] = alloc_batch(0)
                for u in stage_a_units(state[0], 0, prologue=True):
                    u()
            if b + 1 < B:
                state[b + 1] = alloc_batch(b + 1)
                aq.extend(stage_a_units(state[b + 1], b + 1))

            st = state[b]
            qt, ktt, vsb, osb = st["qt"], st["ktt"], st["vsb"], st["osb"]

            for qc in range(NQC):
                o_ps = [
                    psO.tile([65, QC], F32, name=f"o_ps{h}", tag=f"o{h}")
                    for h in range(2)
                ]
                nkb = 4 * qc + 4

                def emit_av(kb, off, n, p_t):
                    for h in range(2):
                        nc.tensor.matmul(
                            o_ps[h][:, off:QC],
                            lhsT=vsb[:, kb, h, 0:65],
                            rhs=p_t[:, h, 0:n],
                            start=(kb == 0),
                            stop=(kb == nkb - 1),
                            skip_group_check=True,
                        )

                pending = []
                for kb in range(nkb):
                    off = max(0, (kb - 4 * qc) * 128)
                    n = QC - off
                    s_ps = psS.tile([128, 2, QC], F32, name="s_ps", tag="s")
                    for h in range(2):
                        nc.tensor.matmul(
                            s_ps[:, h, 0:n],
                            lhsT=ktt[
                                64 * h : 64 * h + 64,
                                kb * 128 : (kb + 1) * 128,
                            ],
                            rhs=qt[
                                64 * h : 64 * h + 64,
                                qc * QC + off : (qc + 1) * QC,
                            ],
                            start=True,
                            stop=True,
                        )
                    p_t = ptp.tile([128, 2, QC], F16, name="p_t", tag="pt")
                    nc.scalar.activation(
                        out=p_t[:, :, 0:n], in_=s_ps[:, :, 0:n], func=Exp
                    )
                    if kb >= 4 * qc:
                        nc.vector.tensor_mul(
                            out=p_t[:, :, 0:128],
                            in0=p_t[:, :, 0:128],
                            in1=tri_sb[:, :]
                            .unsqueeze(1)
                            .broadcast_to([128, 2, 128]),
                        )
                    # filler keeps the PE queue full while the exp for
                    # this block is still in flight
                    pop_filler(kb)
                    pending.append((kb, off, n, p_t))
                    if len(pending) > 3:
                        emit_av(*pending.pop(0))
                for pv in pending:
                    emit_av(*pv)

                # ---- softmax divide: evacuate the accumulators to SBUF
                # right away (releases the PSUM banks so the next query
                # chunk's attn@V can start), then run the reciprocal
                # chain entirely out of SBUF.  The final muls are
                # deferred so they don't head-block the DVE queue while
                # the broadcast is still in flight.
                onum = [
                    smallp.tile([65, QC], F32, name=f"onum{h}", tag=f"on{h}")
                    for h in range(2)
                ]
                for h in range(2):
                    nc.vector.tensor_copy(out=onum[h][:, :], in_=o_ps[h][:, :])
                sp = smallp.tile([32, 32], F32, name="sp", tag="sp")
                nc.gpsimd.dma_start(out=sp[0:16, :], in_=onum[0][64:65, :])
                nc.gpsimd.dma_start(out=sp[16:32, :], in_=onum[1][64:65, :])
                sph = smallp.tile([32, 32], F16, name="sph", tag="sph")
                with nc.allow_low_precision(
                    reason="softmax reciprocal broadcast in fp16 is plenty"
                ):
                    nc.vector.reciprocal(out=sph[:, :], in_=sp[:, :])
                srow = smallp.tile([1, 2, QC], F16, name="srow", tag="srow")
                nc.gpsimd.dma_start(out=srow[0:1, :, :], in_=sph[:, :])
                bch = smallp.tile([64, 2, QC], F16, name="bch", tag="bch")
                nc.gpsimd.partition_broadcast(
                    out_ap=bch[:, :, :], in_ap=srow[0:1, :, :]
                )

                def div_fin(qc=qc, onum=onum, bch=bch, osb=osb):
                    nc.vector.tensor_mul(
                        out=osb[0:64, qc * QC : (qc + 1) * QC],
                        in0=onum[0][0:64, :],
                        in1=bch[:, 0, :],
                    )
                    htmp = smallp.tile([64, QC], F16, name="htmp", tag="htmp")
                    nc.vector.tensor_mul(
                        out=htmp[:, :], in0=onum[1][0:64, :], in1=bch[:, 1, :]
                    )
                    nc.sync.dma_start(
                        out=osb[64:128, qc * QC : (qc + 1) * QC],
                        in_=htmp[:, :],
                    )

                # oproj is deferred TWO query chunks so its ldweights can
                # never reach the PE queue before o_sb is written
                if qc >= 2:
                    lq.append(("op", oproj_unit(st, b, qc - 2, 0)))
                    lq.append(("op", oproj_unit(st, b, qc - 2, 1)))
                lq.append(("df", div_fin))

            # batch end: emit reserved projection units (they cushion the
            # final divide chain), finish lq, then carry the last two
            # query chunks' oproj into the next batch's slots
            leftovers = list(aq)
            aq.clear()
            ncush = min(2, len(leftovers))
            for u in leftovers[: len(leftovers) - ncush]:
                u()
            while lq:
                lq.popleft()[1]()
            for u in leftovers[len(leftovers) - ncush :]:
                u()
            last = b == B - 1
            for qcl in (NQC - 2, NQC - 1):
                lq.append(("op", oproj_unit(st, b, qcl, 0, split_q=last)))
                lq.append(("op", oproj_unit(st, b, qcl, 1, split_q=last)))
            if b == B - 1:
                while lq:
                    lq.popleft()[1]()
            if b - 1 in state:
                del state[b - 1]


def _build(use_bias=False):
    nc = bacc.Bacc("TRN2", target_bir_lowering=False)
    xt = nc.dram_tensor("xt", [128, R // RC, KT, RC], F16, kind="ExternalInput")
    wqkv = nc.dram_tensor("wqkv", [128, KT, 3 * HD], F16, kind="ExternalInput")
    bqkv = nc.dram_tensor("bqkv", [3, HD], F32, kind="ExternalInput")
    wo = nc.dram_tensor("wo", [HD, C], F16, kind="ExternalInput")
    tri = nc.dram_tensor("tri", [128, 128], F16, kind="ExternalInput")
    y = nc.dram_tensor("y", [R, C], F16, kind="ExternalOutput")
    with tile.TileContext(nc) as tc:
        _emit(nc, tc, xt, wqkv, bqkv, wo, tri, y, use_bias)
    nc.finalize()
    return nc


def kernel(hidden_states, w_qkv, b_qkv, w_o, b_o):
    global LAST_RESULT, _CACHED_NC
    X = np.ascontiguousarray(np.asarray(hidden_states, dtype=np.float32)).reshape(
        R, C
    )
    w_qkv = np.asarray(w_qkv, dtype=np.float32)
    b_qkv = np.asarray(b_qkv, dtype=np.float32)
    w_o = np.asarray(w_o, dtype=np.float32)
    b_o = np.asarray(b_o, dtype=np.float32)

    # [ki, rc, ko, col] layout: each partition's per-chunk read is one
    # contiguous 8 KB run, so the x_t DMAs stream at full bandwidth
    Xt = X.T.astype(np.float16).reshape(KT, 128, R // RC, RC)
    Xt = np.ascontiguousarray(Xt.transpose(1, 2, 0, 3))
    scale = float(DH) ** -0.5
    tri_m = np.triu(np.ones((128, 128), dtype=np.float32)).astype(np.float16)

    in_maps = []
    for c in range(NCORES):
        heads = [HPC * c + i for i in range(HPC)]
        wcols, bcols = [], []
        for sec in range(3):  # q, k, v
            sc = scale if sec == 0 else 1.0
            for h in heads:
                lo = sec * C + h * DH
                wcols.append(w_qkv[:, lo : lo + DH] * sc)
                bcols.append(b_qkv[lo : lo + DH] * sc)
        wqkv_c = (
            np.concatenate(wcols, axis=1)
            .astype(np.float16)
            .reshape(KT, 128, 3 * HD)
        )
        wqkv_c = np.ascontiguousarray(wqkv_c.transpose(1, 0, 2))
        bqkv_c = np.ascontiguousarray(np.concatenate(bcols).reshape(3, HD))
        wo_c = np.ascontiguousarray(
            np.concatenate([w_o[h * DH : (h + 1) * DH, :] for h in heads], axis=0)
        ).astype(np.float16)  # [HD, C]
        in_maps.append(
            {
                "xt": Xt,
                "wqkv": wqkv_c,
                "bqkv": bqkv_c,
                "wo": wo_c,
                "tri": tri_m,
            }
        )

    if _CACHED_NC is None:
        _CACHED_NC = _build(use_bias=bool(np.any(b_qkv)))
    res = run_bass_kernel_spmd(_CACHED_NC, in_maps, core_ids=list(range(NCORES)))
    LAST_RESULT = res

    out = res.results[0]["y"].astype(np.float64)
    for c in range(1, NCORES):
        out += res.results[c]["y"]
    out += b_o
    return out.astype(np.float32).reshape(B, T, C)
